# revision 27
# baseline (speedup 1.0000x reference)
"""Trainium2 Bass kernel for nn_GumbelLinear (topk_masking).

Computation:
  h (64,16) -> conditional range-remap (global min/max of h) ->
  mask = h @ w_p + bias -> logits = mask + g1 - g2 (Gumbel noise from
  U1/U2) -> per-row top-5 hard mask (straight-through).

Sharding: replicate h (needed for the global min/max) and w_p; data-parallel
the 64-row axis across 8 cores (8 rows each).  Host side only reshapes /
transposes / slices / concatenates numpy arrays; all math runs on device.

Device notes:
  - ONE packed [16,136] input tensor -> a single DMA.  The DMA issue, the
    eps memset + dummy Ln (ACT table preload) are surgically moved BEFORE
    the bass-init all-engine barrier so the ~2us DMA completion latency and
    the ~1.3us ACT table load overlap the fixed engine-preamble phase.
    Engines that read the packed tile wait on a manual DMA semaphore.
  - The reference's clip((h-min)/(max-min)*0.6-0.3, -.3, .3) never binds
    (the normalized value is in [-0.3,0.3] by construction), so the remap
    is affine in h.  Split matmul: logits = (h@w + base) + s*(dlt@w) with
    dlt = (h+mneg)*alpha - 0.3 - h, s = (max(gmax,mneg)>100).  pm0 = h@w
    runs as soon as the DMA lands (PE is idle), so only the small dlt@w
    matmul sits behind the global-reduce chain.  For in-range data s=0
    gives logits = pm0 + base bit-exactly.
  - Global max/-min: per-partition X-reduce into two columns, 32x32
    stream-transpose, one X-reduce over both rows, two stream-shuffle
    broadcasts (partition 0/1 -> all).
  - sigmoid is monotonic, so the top-5 threshold compare runs on logits
    directly; the straight-through output equals the 0/1 mask itself.
"""

import numpy as np

N_CORES = 8
ROWS = 64
D = 16
RPC = ROWS // N_CORES  # rows per core
EPS = 1e-8

# packed layout, one [16, 136] f32 tensor:
C_HT = 0       # [0:16, 0:64]    h transposed (full, replicated)
C_HTS = 64     # [0:16, 64:72]   this core's 8 rows of h, transposed
C_WP = 72      # [0:16, 72:88]   w_p
C_B = 88       # [0:8, 88:104]   bias rows
C_U1 = 104     # [0:8, 104:120]  U1 rows (flattened)
C_U2 = 120     # [0:8, 120:136]  U2 rows (flattened)
C_END = 136

_CACHE = {}


def _build_nc():
    import concourse.tile as tile
    from concourse import bacc, mybir
    from concourse.tile_rust import add_dep_helper

    f32 = mybir.dt.float32
    Alu = mybir.AluOpType
    Act = mybir.ActivationFunctionType

    nc = bacc.Bacc(
        "TRN2", debug=False, enable_asserts=False, enable_partition_id=False
    )

    packed = nc.dram_tensor("packed", (D, C_END), f32, kind="ExternalInput")
    out_s = nc.dram_tensor("out_s", (RPC, D), f32, kind="ExternalOutput")

    entry = nc.main_func.blocks[0].instructions
    init_len = len(entry)

    # ---- pre-TileContext region ----------------------------------------
    # raw tensors + manual sems; the hoist below moves the tagged
    # instructions before the bass-init all-engine barrier.
    tin = nc.alloc_sbuf_tensor("tin", [D, C_END], f32)
    eps_raw = nc.alloc_sbuf_tensor("eps_raw", [RPC, 1], f32)
    dscr = nc.alloc_sbuf_tensor("dscr", [1, 1], f32)
    NEG = -1.0e30
    scr_raw = nc.alloc_sbuf_tensor("scr_raw", [32, 33], f32)
    scrT_raw = nc.alloc_sbuf_tensor("scrT_raw", [32, 33], f32)
    dma_sem = nc.alloc_semaphore("in_dma_sem")
    eps_sem = nc.alloc_semaphore("eps_sem")

    # hoisted before the init barrier: the input DMA issues from SP (the
    # land time is floor-bound by the boot phase either way, and keeping
    # ACT free lets both ACT table loads finish before the data lands);
    # the dep-free memsets run on DVE.
    hoist = []
    hoist.append(
        nc.sync.dma_start(tin.ap(), packed[:]).then_inc(dma_sem, 16).ins
    )
    hoist.append(nc.vector.memset(scr_raw.ap(), NEG).ins)
    hoist.append(nc.vector.memset(scrT_raw.ap(), NEG).ins)
    hoist.append(nc.vector.memset(eps_raw.ap(), EPS).then_inc(eps_sem, 1).ins)
    # dummy Ln anchors the (single) ACT table load before the barrier, so
    # the first real ACTIVATE can start the moment the input DMA lands
    hoist.append(nc.scalar.wait_ge(eps_sem, 1).ins)
    hoist.append(
        nc.scalar.activation(
            dscr.ap(), eps_raw.ap()[0:1, 0:1], Act.Ln,
            bias=eps_raw.ap()[0:1, :], scale=1.0,
        ).ins
    )

    # engine gates for the manual DMA (stay after the init barrier)
    nc.vector.wait_ge(dma_sem, 16)
    nc.scalar.wait_ge(dma_sem, 16)
    nc.tensor.wait_ge(dma_sem, 16)

    t = tin.ap()
    v_hT = t[:, C_HT:C_HTS]
    v_hTs = t[:, C_HTS:C_WP]
    v_wp = t[:, C_WP:C_B]
    v_bias = t[0:RPC, C_B:C_U1]
    v_u1 = t[0:RPC, C_U1:C_U2]
    v_u2 = t[0:RPC, C_U2:C_END]
    eps_b = eps_raw.ap()

    with tile.TileContext(nc) as tc:
        with (
            tc.tile_pool(name="sb", bufs=1) as sb,
            tc.tile_pool(name="ps", bufs=1, space=tile.bass.MemorySpace.PSUM) as ps,
        ):
            # ---- pm0 = h.T @ wp immediately (PE idle until now) ----
            pm0 = ps.tile([RPC, D], f32)
            nc.tensor.matmul(pm0[:], v_hTs, v_wp, start=True, stop=True)

            # ---- global max / -min of h, broadcast to all partitions ----
            # scr/scrT are raw tensors NEG-filled pre-barrier; in-TC deps
            # between the reduce/transpose ops are tracked via shadow memory.
            scr = scr_raw.ap()
            scrT = scrT_raw.ap()
            nc.vector.tensor_reduce(
                scr[0:D, 0:1], v_hT, axis=mybir.AxisListType.X, op=Alu.max
            )
            nc.vector.tensor_reduce(
                scr[0:D, 1:2], v_hT, axis=mybir.AxisListType.X, op=Alu.min,
                negate=True,
            )
            nc.vector.transpose(scrT[:, 0:32], scr[:, 0:32])
            # scrT row 0 = per-column maxes, row 1 = negated per-column mins
            nc.vector.tensor_reduce(
                scrT[0:2, 32:33], scrT[0:2, 0:32], axis=mybir.AxisListType.X,
                op=Alu.max,
            )
            bc = sb.tile([32, 2], f32)
            nc.vector.stream_shuffle(bc[:, 0:1], scrT[:, 32:33], mask=[0] * 32)
            nc.vector.stream_shuffle(bc[:, 1:2], scrT[:, 32:33], mask=[1] * 32)
            gmax = bc[0:D, 0:1]  # max(h) on every partition
            mneg = bc[0:D, 1:2]  # -min(h) on every partition

            # alpha = 0.6/(gmax+mneg)
            r1 = sb.tile([D, 1], f32)
            nc.vector.tensor_scalar(
                r1[:], gmax, mneg, 1.0 / 0.6, op0=Alu.add, op1=Alu.mult
            )
            alpha = sb.tile([D, 1], f32)
            nc.vector.reciprocal(alpha[:], r1[:])

            # dlt = (h+mneg)*alpha - 0.3 - h   (mapped minus h)
            p = sb.tile([D, RPC], f32)
            nc.vector.tensor_scalar(
                p[:], v_hTs, mneg, alpha[:], op0=Alu.add, op1=Alu.mult
            )
            dlt = sb.tile([D, RPC], f32)
            i_dlt = nc.vector.scalar_tensor_tensor(
                dlt[:], in0=p[:], scalar=0.3, in1=v_hTs,
                op0=Alu.subtract, op1=Alu.subtract,
            )

            # s = (max(gmax, mneg) > 100) fused into one tensor_scalar.
            # Ordered after dlt (nosync) so it fills the dw-matmul bubble
            # instead of delaying the alpha chain.
            s = sb.tile([D, 1], f32)
            i_s = nc.vector.tensor_scalar(
                s[:], gmax, mneg, 100.0, op0=Alu.max, op1=Alu.is_gt
            )
            add_dep_helper(i_s.ins, i_dlt.ins, sync=False)

            # ---- dw = dlt.T @ wp (correction matmul) ----
            dw = ps.tile([RPC, D], f32)
            nc.tensor.matmul(dw[:], dlt[:], v_wp, start=True, stop=True)

            # ---- Gumbel: b = ln(-ln(U + eps) + eps); g = -b (ACT) ----
            a1 = sb.tile([RPC, D], f32)
            nc.scalar.activation(a1[:], v_u1, Act.Ln, bias=eps_b, scale=1.0)
            a2 = sb.tile([RPC, D], f32)
            nc.scalar.activation(a2[:], v_u2, Act.Ln, bias=eps_b, scale=1.0)
            b1 = sb.tile([RPC, D], f32)
            nc.scalar.activation(b1[:], a1[:], Act.Ln, bias=eps_b, scale=-1.0)
            b2 = sb.tile([RPC, D], f32)
            nc.scalar.activation(b2[:], a2[:], Act.Ln, bias=eps_b, scale=-1.0)

            # base = bias + g1 - g2 = bias - b1 + b2.  Ordered after `dlt`
            # (nosync dep) so these fill the DVE bubble during the dw matmul
            # instead of delaying the critical chain.
            gg = sb.tile([RPC, D], f32)
            i_gg = nc.vector.tensor_sub(gg[:], b2[:], b1[:])
            add_dep_helper(i_gg.ins, i_dlt.ins, sync=False)
            base = sb.tile([RPC, D], f32)
            nc.vector.tensor_add(base[:], gg[:], v_bias)

            # logits = (s*dw + base) + pm0; sigmoid is monotonic so the
            # top-5 threshold compare runs on logits directly
            l1 = sb.tile([RPC, D], f32)
            nc.vector.scalar_tensor_tensor(
                l1[:], in0=dw[:], scalar=s[0:RPC, :], in1=base[:],
                op0=Alu.mult, op1=Alu.add,
            )
            logits = sb.tile([RPC, D], f32)
            nc.vector.tensor_add(logits[:], l1[:], pm0[:])
            top8 = sb.tile([RPC, 8], f32)
            nc.vector.max(top8[:], logits[:])
            hard = sb.tile([RPC, D], f32)
            nc.vector.tensor_scalar(
                hard[:], logits[:], top8[:, 4:5], None, op0=Alu.is_ge
            )

            i_out = nc.sync.dma_start(out_s[:], hard[:])

    # ---- overlap the out-DMA completion with the first TC-exit barrier --
    # TC exit emits: SP drain (waits all sems incl the out-DMA's) ->
    # barrier -> Pool reset-drain (waits the DMA again via its reset
    # range) + range-clear -> barrier.  Dropping the out-DMA wait from the
    # SP drain lets barrier 1 run during the ~1.4us HBM write receipt; the
    # Pool reset-drain still enforces completion before the clear.
    end_blk = next(b for b in nc.main_func.blocks if b.name.endswith("_end"))
    out_sem_id = next(
        u.id for u in i_out.ins.sync_info.on_update if u.update_value == 16
    )
    sp_drain = next(
        i for i in end_blk.instructions
        if type(i).__name__ == "InstDrain" and str(i.engine).endswith("SP")
    )
    pool_drain = next(
        i for i in end_blk.instructions
        if type(i).__name__ == "InstDrain" and i.is_reset_sema
    )
    assert pool_drain.reset_range_start <= out_sem_id < pool_drain.reset_range_stop
    old_waits = sp_drain.sync_info.on_wait
    new_waits = [x for x in old_waits if x.id != out_sem_id]
    assert len(new_waits) == len(old_waits) - 1, (out_sem_id, old_waits)
    sp_drain.sync_info.on_wait = new_waits

    # restore the manual semaphores so the NEFF is safely re-executable
    sem_lo = min(dma_sem.num, eps_sem.num)
    sem_hi = max(dma_sem.num, eps_sem.num)
    nc.gpsimd.dma_reset(range(sem_lo, sem_hi + 1))
    nc.gpsimd.sem_clear(range(sem_lo, sem_hi + 1))

    # ---- hoist the tagged pre-TC instructions into the engine preambles ---
    # each engine's init preamble ends with its bcreg1_hi register move;
    # inserting right after it puts the instruction before the codegen's
    # second sync point, so the DMA issue/memsets overlap the fixed
    # engine-boot phase.
    hoist_insts = list(hoist)
    idx = {id(inst): k for k, inst in enumerate(entry)}
    positions = sorted(idx[id(inst)] for inst in hoist_insts)
    for pos in reversed(positions):
        del entry[pos]

    def preamble_end(engine_prefix):
        for k, ins in enumerate(entry[:init_len]):
            if (
                type(ins).__name__ == "InstRegisterMove"
                and f"{engine_prefix}_bcreg1_hi" in str(ins)
            ):
                return k + 1
        raise RuntimeError(f"no preamble end for {engine_prefix}")

    by_engine = {}
    for inst in hoist_insts:
        by_engine.setdefault(str(inst.engine), []).append(inst)
    targets = []
    for eng_name, insts in by_engine.items():
        prefix = eng_name.split(".")[-1]  # EngineType.Activation -> Activation
        targets.append((preamble_end(prefix), insts))
    for pos, insts in sorted(targets, reverse=True):
        for inst in reversed(insts):
            entry.insert(pos, inst)

    nc.compile()
    return nc


def _get_nc():
    if "nc" not in _CACHE:
        _CACHE["nc"] = _build_nc()
    return _CACHE["nc"]


def _make_in_maps(h, w_p, bias, U1, U2):
    h = np.ascontiguousarray(np.asarray(h, np.float32).reshape(ROWS, D))
    hT = h.T
    wp = np.asarray(w_p, np.float32)
    bias = np.asarray(bias, np.float32).reshape(ROWS, D)
    u1 = np.asarray(U1, np.float32).reshape(ROWS, D)
    u2 = np.asarray(U2, np.float32).reshape(ROWS, D)

    in_maps = []
    for c in range(N_CORES):
        rows = slice(c * RPC, (c + 1) * RPC)
        pa = np.full((D, C_END), 0.5, np.float32)
        pa[:, C_HT:C_HTS] = hT
        pa[:, C_HTS:C_WP] = h[rows].T
        pa[:, C_WP:C_B] = wp
        pa[0:RPC, C_B:C_U1] = bias[rows]
        pa[0:RPC, C_U1:C_U2] = u1[rows]
        pa[0:RPC, C_U2:C_END] = u2[rows]
        in_maps.append({"packed": pa})
    return in_maps


def kernel(h, input, w_p, bias, U1, U2, **_unused):
    from concourse.bass_utils import run_bass_kernel_spmd

    nc = _get_nc()
    in_maps = _make_in_maps(h, w_p, bias, U1, U2)
    res = run_bass_kernel_spmd(nc, in_maps, core_ids=list(range(N_CORES)))
    out = np.concatenate([r["out_s"] for r in res.results], axis=0)
    return out.reshape(ROWS, 4, 4).astype(np.float32)


# revision 28
# speedup vs baseline: 1.1796x; 1.1796x over previous
"""Trainium2 Bass kernel for nn_GumbelLinear (topk_masking).

Computation:
  h (64,16) -> conditional range-remap (global min/max of h) ->
  mask = h @ w_p + bias -> logits = mask + g1 - g2 (Gumbel noise from
  U1/U2) -> per-row top-5 hard mask (straight-through).

Sharding: replicate h (needed for the global min/max) and w_p; data-parallel
the 64-row axis across 8 cores (8 rows each).  Host side only reshapes /
transposes / slices / concatenates numpy arrays; all math runs on device.

Device notes:
  - ONE packed [16,136] input tensor -> a single DMA.  The DMA issue, the
    eps memset + dummy Ln (ACT table preload) are surgically moved BEFORE
    the bass-init all-engine barrier so the ~2us DMA completion latency and
    the ~1.3us ACT table load overlap the fixed engine-preamble phase.
    Engines that read the packed tile wait on a manual DMA semaphore.
  - The reference's clip((h-min)/(max-min)*0.6-0.3, -.3, .3) never binds
    (the normalized value is in [-0.3,0.3] by construction), so the remap
    is affine in h.  Split matmul: logits = (h@w + base) + s*(dlt@w) with
    dlt = (h+mneg)*alpha - 0.3 - h, s = (max(gmax,mneg)>100).  pm0 = h@w
    runs as soon as the DMA lands (PE is idle), so only the small dlt@w
    matmul sits behind the global-reduce chain.  For in-range data s=0
    gives logits = pm0 + base bit-exactly.
  - Global max/-min: per-partition X-reduce into two columns, 32x32
    stream-transpose, one X-reduce over both rows, two stream-shuffle
    broadcasts (partition 0/1 -> all).
  - sigmoid is monotonic, so the top-5 threshold compare runs on logits
    directly; the straight-through output equals the 0/1 mask itself.
"""

import numpy as np

N_CORES = 8
ROWS = 64
D = 16
RPC = ROWS // N_CORES  # rows per core
EPS = 1e-8

# packed layout, one [16, 136] f32 tensor:
C_HT = 0       # [0:16, 0:64]    h transposed (full, replicated)
C_HTS = 64     # [0:16, 64:72]   this core's 8 rows of h, transposed
C_WP = 72      # [0:16, 72:88]   w_p
C_B = 88       # [0:8, 88:104]   bias rows
C_U1 = 104     # [0:8, 104:120]  U1 rows (flattened)
C_U2 = 120     # [0:8, 120:136]  U2 rows (flattened)
C_END = 136

_CACHE = {}


def _build_nc():
    import concourse.tile as tile
    from concourse import bacc, mybir
    from concourse.tile_rust import add_dep_helper

    f32 = mybir.dt.float32
    Alu = mybir.AluOpType
    Act = mybir.ActivationFunctionType

    nc = bacc.Bacc(
        "TRN2", debug=False, enable_asserts=False, enable_partition_id=False
    )

    packed = nc.dram_tensor("packed", (D, C_END), f32, kind="ExternalInput")
    out_s = nc.dram_tensor("out_s", (RPC, D), f32, kind="ExternalOutput")

    entry = nc.main_func.blocks[0].instructions
    init_len = len(entry)

    # ---- pre-TileContext region ----------------------------------------
    # raw tensors + manual sems; the hoist below moves the tagged
    # instructions before the bass-init all-engine barrier.
    tin = nc.alloc_sbuf_tensor("tin", [D, C_END], f32)
    eps_raw = nc.alloc_sbuf_tensor("eps_raw", [RPC, 1], f32)
    dscr = nc.alloc_sbuf_tensor("dscr", [1, 1], f32)
    NEG = -1.0e30
    scr_raw = nc.alloc_sbuf_tensor("scr_raw", [32, 33], f32)
    scrT_raw = nc.alloc_sbuf_tensor("scrT_raw", [32, 33], f32)
    dma_sem = nc.alloc_semaphore("in_dma_sem")
    eps_sem = nc.alloc_semaphore("eps_sem")

    # hoisted before the init barrier: the input DMA issues from SP (the
    # land time is floor-bound by the boot phase either way, and keeping
    # ACT free lets both ACT table loads finish before the data lands);
    # the dep-free memsets run on DVE.
    hoist = []
    hoist.append(
        nc.sync.dma_start(tin.ap(), packed[:]).then_inc(dma_sem, 16).ins
    )
    hoist.append(nc.vector.memset(scr_raw.ap(), NEG).ins)
    hoist.append(nc.vector.memset(scrT_raw.ap(), NEG).ins)
    hoist.append(nc.vector.memset(eps_raw.ap(), EPS).then_inc(eps_sem, 1).ins)
    # dummy Ln anchors the (single) ACT table load before the barrier, so
    # the first real ACTIVATE can start the moment the input DMA lands
    hoist.append(nc.scalar.wait_ge(eps_sem, 1).ins)
    hoist.append(
        nc.scalar.activation(
            dscr.ap(), eps_raw.ap()[0:1, 0:1], Act.Ln,
            bias=eps_raw.ap()[0:1, :], scale=1.0,
        ).ins
    )

    # engine gates for the manual DMA (stay after the init barrier)
    nc.vector.wait_ge(dma_sem, 16)
    nc.scalar.wait_ge(dma_sem, 16)
    nc.tensor.wait_ge(dma_sem, 16)

    t = tin.ap()
    v_hT = t[:, C_HT:C_HTS]
    v_hTs = t[:, C_HTS:C_WP]
    v_wp = t[:, C_WP:C_B]
    v_bias = t[0:RPC, C_B:C_U1]
    v_u1 = t[0:RPC, C_U1:C_U2]
    v_u2 = t[0:RPC, C_U2:C_END]
    eps_b = eps_raw.ap()

    with tile.TileContext(nc) as tc:
        with (
            tc.tile_pool(name="sb", bufs=1) as sb,
            tc.tile_pool(name="ps", bufs=1, space=tile.bass.MemorySpace.PSUM) as ps,
        ):
            # ---- pm0 = h.T @ wp immediately (PE idle until now) ----
            pm0 = ps.tile([RPC, D], f32)
            nc.tensor.matmul(pm0[:], v_hTs, v_wp, start=True, stop=True)

            # ---- global max / -min of h, broadcast to all partitions ----
            # scr/scrT are raw tensors NEG-filled pre-barrier; in-TC deps
            # between the reduce/transpose ops are tracked via shadow memory.
            scr = scr_raw.ap()
            scrT = scrT_raw.ap()
            nc.vector.tensor_reduce(
                scr[0:D, 0:1], v_hT, axis=mybir.AxisListType.X, op=Alu.max
            )
            nc.vector.tensor_reduce(
                scr[0:D, 1:2], v_hT, axis=mybir.AxisListType.X, op=Alu.min,
                negate=True,
            )
            nc.vector.transpose(scrT[:, 0:32], scr[:, 0:32])
            # scrT row 0 = per-column maxes, row 1 = negated per-column mins
            nc.vector.tensor_reduce(
                scrT[0:2, 32:33], scrT[0:2, 0:32], axis=mybir.AxisListType.X,
                op=Alu.max,
            )
            bc = sb.tile([32, 2], f32)
            nc.vector.stream_shuffle(bc[:, 0:1], scrT[:, 32:33], mask=[0] * 32)
            nc.vector.stream_shuffle(bc[:, 1:2], scrT[:, 32:33], mask=[1] * 32)
            gmax = bc[0:D, 0:1]  # max(h) on every partition
            mneg = bc[0:D, 1:2]  # -min(h) on every partition

            # alpha = 0.6/(gmax+mneg)
            r1 = sb.tile([D, 1], f32)
            nc.vector.tensor_scalar(
                r1[:], gmax, mneg, 1.0 / 0.6, op0=Alu.add, op1=Alu.mult
            )
            alpha = sb.tile([D, 1], f32)
            nc.vector.reciprocal(alpha[:], r1[:])

            # dlt = (h+mneg)*alpha - 0.3 - h   (mapped minus h)
            p = sb.tile([D, RPC], f32)
            nc.vector.tensor_scalar(
                p[:], v_hTs, mneg, alpha[:], op0=Alu.add, op1=Alu.mult
            )
            dlt = sb.tile([D, RPC], f32)
            i_dlt = nc.vector.scalar_tensor_tensor(
                dlt[:], in0=p[:], scalar=0.3, in1=v_hTs,
                op0=Alu.subtract, op1=Alu.subtract,
            )

            # s = (max(gmax, mneg) > 100) fused into one tensor_scalar.
            # Ordered after dlt (nosync) so it fills the dw-matmul bubble
            # instead of delaying the alpha chain.
            s = sb.tile([D, 1], f32)
            i_s = nc.vector.tensor_scalar(
                s[:], gmax, mneg, 100.0, op0=Alu.max, op1=Alu.is_gt
            )
            add_dep_helper(i_s.ins, i_dlt.ins, sync=False)

            # ---- dw = dlt.T @ wp (correction matmul) ----
            dw = ps.tile([RPC, D], f32)
            nc.tensor.matmul(dw[:], dlt[:], v_wp, start=True, stop=True)

            # ---- Gumbel: b = ln(-ln(U + eps) + eps); g = -b (ACT) ----
            a1 = sb.tile([RPC, D], f32)
            nc.scalar.activation(a1[:], v_u1, Act.Ln, bias=eps_b, scale=1.0)
            a2 = sb.tile([RPC, D], f32)
            nc.scalar.activation(a2[:], v_u2, Act.Ln, bias=eps_b, scale=1.0)
            b1 = sb.tile([RPC, D], f32)
            nc.scalar.activation(b1[:], a1[:], Act.Ln, bias=eps_b, scale=-1.0)
            b2 = sb.tile([RPC, D], f32)
            nc.scalar.activation(b2[:], a2[:], Act.Ln, bias=eps_b, scale=-1.0)

            # base = bias + g1 - g2 = bias - b1 + b2.  Ordered after `dlt`
            # (nosync dep) so these fill the DVE bubble during the dw matmul
            # instead of delaying the critical chain.
            gg = sb.tile([RPC, D], f32)
            i_gg = nc.vector.tensor_sub(gg[:], b2[:], b1[:])
            add_dep_helper(i_gg.ins, i_dlt.ins, sync=False)
            base = sb.tile([RPC, D], f32)
            nc.vector.tensor_add(base[:], gg[:], v_bias)
            # l0 = base + pm0 overlaps the dw matmul tail (pm0 is long
            # done), so only the final STT waits on the dw semaphore
            l0 = sb.tile([RPC, D], f32)
            nc.vector.tensor_add(l0[:], base[:], pm0[:])

            # logits = s*dw + l0; sigmoid is monotonic so the top-5
            # threshold compare runs on logits directly
            logits = sb.tile([RPC, D], f32)
            nc.vector.scalar_tensor_tensor(
                logits[:], in0=dw[:], scalar=s[0:RPC, :], in1=l0[:],
                op0=Alu.mult, op1=Alu.add,
            )
            top8 = sb.tile([RPC, 8], f32)
            nc.vector.max(top8[:], logits[:])
            hard = sb.tile([RPC, D], f32)
            nc.vector.tensor_scalar(
                hard[:], logits[:], top8[:, 4:5], None, op0=Alu.is_ge
            )

            i_out = nc.sync.dma_start(out_s[:], hard[:])

    # ---- overlap the out-DMA completion with the first TC-exit barrier --
    # TC exit emits: SP drain (waits all sems incl the out-DMA's) ->
    # barrier -> Pool reset-drain (waits the DMA again via its reset
    # range) + range-clear -> barrier.  Dropping the out-DMA wait from the
    # SP drain lets barrier 1 run during the ~1.4us HBM write receipt; the
    # Pool reset-drain still enforces completion before the clear.
    end_blk = next(b for b in nc.main_func.blocks if b.name.endswith("_end"))
    out_sem_id = next(
        u.id for u in i_out.ins.sync_info.on_update if u.update_value == 16
    )
    sp_drain = next(
        i for i in end_blk.instructions
        if type(i).__name__ == "InstDrain" and str(i.engine).endswith("SP")
    )
    pool_drain = next(
        i for i in end_blk.instructions
        if type(i).__name__ == "InstDrain" and i.is_reset_sema
    )
    assert pool_drain.reset_range_start <= out_sem_id < pool_drain.reset_range_stop
    old_waits = sp_drain.sync_info.on_wait
    new_waits = [x for x in old_waits if x.id != out_sem_id]
    assert len(new_waits) == len(old_waits) - 1, (out_sem_id, old_waits)
    sp_drain.sync_info.on_wait = new_waits

    # restore the manual semaphores so the NEFF is safely re-executable
    sem_lo = min(dma_sem.num, eps_sem.num)
    sem_hi = max(dma_sem.num, eps_sem.num)
    nc.gpsimd.dma_reset(range(sem_lo, sem_hi + 1))
    nc.gpsimd.sem_clear(range(sem_lo, sem_hi + 1))

    # ---- hoist the tagged pre-TC instructions into the engine preambles ---
    # each engine's init preamble ends with its bcreg1_hi register move;
    # inserting right after it puts the instruction before the codegen's
    # second sync point, so the DMA issue/memsets overlap the fixed
    # engine-boot phase.
    hoist_insts = list(hoist)
    idx = {id(inst): k for k, inst in enumerate(entry)}
    positions = sorted(idx[id(inst)] for inst in hoist_insts)
    for pos in reversed(positions):
        del entry[pos]

    def preamble_end(engine_prefix):
        for k, ins in enumerate(entry[:init_len]):
            if (
                type(ins).__name__ == "InstRegisterMove"
                and f"{engine_prefix}_bcreg1_hi" in str(ins)
            ):
                return k + 1
        raise RuntimeError(f"no preamble end for {engine_prefix}")

    by_engine = {}
    for inst in hoist_insts:
        by_engine.setdefault(str(inst.engine), []).append(inst)
    targets = []
    for eng_name, insts in by_engine.items():
        prefix = eng_name.split(".")[-1]  # EngineType.Activation -> Activation
        targets.append((preamble_end(prefix), insts))
    for pos, insts in sorted(targets, reverse=True):
        for inst in reversed(insts):
            entry.insert(pos, inst)

    nc.compile()
    return nc


def _get_nc():
    if "nc" not in _CACHE:
        _CACHE["nc"] = _build_nc()
    return _CACHE["nc"]


def _make_in_maps(h, w_p, bias, U1, U2):
    h = np.ascontiguousarray(np.asarray(h, np.float32).reshape(ROWS, D))
    hT = h.T
    wp = np.asarray(w_p, np.float32)
    bias = np.asarray(bias, np.float32).reshape(ROWS, D)
    u1 = np.asarray(U1, np.float32).reshape(ROWS, D)
    u2 = np.asarray(U2, np.float32).reshape(ROWS, D)

    in_maps = []
    for c in range(N_CORES):
        rows = slice(c * RPC, (c + 1) * RPC)
        pa = np.full((D, C_END), 0.5, np.float32)
        pa[:, C_HT:C_HTS] = hT
        pa[:, C_HTS:C_WP] = h[rows].T
        pa[:, C_WP:C_B] = wp
        pa[0:RPC, C_B:C_U1] = bias[rows]
        pa[0:RPC, C_U1:C_U2] = u1[rows]
        pa[0:RPC, C_U2:C_END] = u2[rows]
        in_maps.append({"packed": pa})
    return in_maps


def kernel(h, input, w_p, bias, U1, U2, **_unused):
    from concourse.bass_utils import run_bass_kernel_spmd

    nc = _get_nc()
    in_maps = _make_in_maps(h, w_p, bias, U1, U2)
    res = run_bass_kernel_spmd(nc, in_maps, core_ids=list(range(N_CORES)))
    out = np.concatenate([r["out_s"] for r in res.results], axis=0)
    return out.reshape(ROWS, 4, 4).astype(np.float32)


# revision 31
# speedup vs baseline: 1.1813x; 1.0014x over previous
"""Trainium2 Bass kernel for nn_GumbelLinear (topk_masking).

Computation:
  h (64,16) -> conditional range-remap (global min/max of h) ->
  mask = h @ w_p + bias -> logits = mask + g1 - g2 (Gumbel noise from
  U1/U2) -> per-row top-5 hard mask (straight-through).

Sharding: replicate h (needed for the global min/max) and w_p; data-parallel
the 64-row axis across 8 cores (8 rows each).  Host side only reshapes /
transposes / slices / concatenates numpy arrays; all math runs on device.

Device notes:
  - ONE packed [16,136] input tensor -> a single DMA.  The DMA issue, the
    eps memset + dummy Ln (ACT table preload) are surgically moved BEFORE
    the bass-init all-engine barrier so the ~2us DMA completion latency and
    the ~1.3us ACT table load overlap the fixed engine-preamble phase.
    Engines that read the packed tile wait on a manual DMA semaphore.
  - The reference's clip((h-min)/(max-min)*0.6-0.3, -.3, .3) never binds
    (the normalized value is in [-0.3,0.3] by construction), so the remap
    is affine in h.  Split matmul: logits = (h@w + base) + s*(dlt@w) with
    dlt = (h+mneg)*alpha - 0.3 - h, s = (max(gmax,mneg)>100).  pm0 = h@w
    runs as soon as the DMA lands (PE is idle), so only the small dlt@w
    matmul sits behind the global-reduce chain.  For in-range data s=0
    gives logits = pm0 + base bit-exactly.
  - Global max/-min: per-partition X-reduce into two columns, 32x32
    stream-transpose, one X-reduce over both rows, two stream-shuffle
    broadcasts (partition 0/1 -> all).
  - sigmoid is monotonic, so the top-5 threshold compare runs on logits
    directly; the straight-through output equals the 0/1 mask itself.
"""

import numpy as np

N_CORES = 8
ROWS = 64
D = 16
RPC = ROWS // N_CORES  # rows per core
EPS = 1e-8

# packed layout, one [16, 136] f32 tensor:
C_HT = 0       # [0:16, 0:64]    h transposed (full, replicated)
C_HTS = 64     # [0:16, 64:72]   this core's 8 rows of h, transposed
C_WP = 72      # [0:16, 72:88]   w_p
C_B = 88       # [0:8, 88:104]   bias rows
C_U1 = 104     # [0:8, 104:120]  U1 rows (flattened)
C_U2 = 120     # [0:8, 120:136]  U2 rows (flattened)
C_END = 136

_CACHE = {}


def _build_nc():
    import concourse.tile as tile
    from concourse import bacc, mybir
    from concourse.tile_rust import add_dep_helper

    f32 = mybir.dt.float32
    Alu = mybir.AluOpType
    Act = mybir.ActivationFunctionType

    nc = bacc.Bacc(
        "TRN2", debug=False, enable_asserts=False, enable_partition_id=False
    )

    packed = nc.dram_tensor("packed", (D, C_END), f32, kind="ExternalInput")
    out_s = nc.dram_tensor("out_s", (RPC, D), f32, kind="ExternalOutput")

    entry = nc.main_func.blocks[0].instructions
    init_len = len(entry)

    # ---- pre-TileContext region ----------------------------------------
    # raw tensors + manual sems; the hoist below moves the tagged
    # instructions before the bass-init all-engine barrier.
    tin = nc.alloc_sbuf_tensor("tin", [D, C_END], f32)
    eps_raw = nc.alloc_sbuf_tensor("eps_raw", [RPC, 1], f32)
    dscr = nc.alloc_sbuf_tensor("dscr", [1, 1], f32)
    NEG = -1.0e30
    scr_raw = nc.alloc_sbuf_tensor("scr_raw", [32, 33], f32)
    scrT_raw = nc.alloc_sbuf_tensor("scrT_raw", [32, 33], f32)
    dma_sem = nc.alloc_semaphore("in_dma_sem")
    eps_sem = nc.alloc_semaphore("eps_sem")

    # hoisted before the init barrier: the input DMA issues from SP (the
    # land time is floor-bound by the boot phase either way, and keeping
    # ACT free lets both ACT table loads finish before the data lands);
    # the dep-free memsets run on DVE.
    hoist = []
    hoist.append(
        nc.sync.dma_start(tin.ap(), packed[:], single_packet=True)
        .then_inc(dma_sem, 16)
        .ins
    )
    hoist.append(nc.vector.memset(scr_raw.ap(), NEG).ins)
    hoist.append(nc.vector.memset(scrT_raw.ap(), NEG).ins)
    hoist.append(nc.vector.memset(eps_raw.ap(), EPS).then_inc(eps_sem, 1).ins)
    # dummy Ln anchors the (single) ACT table load before the barrier, so
    # the first real ACTIVATE can start the moment the input DMA lands
    hoist.append(nc.scalar.wait_ge(eps_sem, 1).ins)
    hoist.append(
        nc.scalar.activation(
            dscr.ap(), eps_raw.ap()[0:1, 0:1], Act.Ln,
            bias=eps_raw.ap()[0:1, :], scale=1.0,
        ).ins
    )

    # engine gates for the manual DMA (stay after the init barrier)
    nc.vector.wait_ge(dma_sem, 16)
    nc.scalar.wait_ge(dma_sem, 16)
    nc.tensor.wait_ge(dma_sem, 16)

    t = tin.ap()
    v_hT = t[:, C_HT:C_HTS]
    v_hTs = t[:, C_HTS:C_WP]
    v_wp = t[:, C_WP:C_B]
    v_bias = t[0:RPC, C_B:C_U1]
    v_u1 = t[0:RPC, C_U1:C_U2]
    v_u2 = t[0:RPC, C_U2:C_END]
    eps_b = eps_raw.ap()

    with tile.TileContext(nc) as tc:
        with (
            tc.tile_pool(name="sb", bufs=1) as sb,
            tc.tile_pool(name="ps", bufs=1, space=tile.bass.MemorySpace.PSUM) as ps,
        ):
            # ---- pm0 = h.T @ wp immediately (PE idle until now) ----
            pm0 = ps.tile([RPC, D], f32)
            nc.tensor.matmul(pm0[:], v_hTs, v_wp, start=True, stop=True)

            # ---- global max / -min of h, broadcast to all partitions ----
            # scr/scrT are raw tensors NEG-filled pre-barrier; in-TC deps
            # between the reduce/transpose ops are tracked via shadow memory.
            scr = scr_raw.ap()
            scrT = scrT_raw.ap()
            nc.vector.tensor_reduce(
                scr[0:D, 0:1], v_hT, axis=mybir.AxisListType.X, op=Alu.max
            )
            nc.vector.tensor_reduce(
                scr[0:D, 1:2], v_hT, axis=mybir.AxisListType.X, op=Alu.min,
                negate=True,
            )
            nc.vector.transpose(scrT[:, 0:32], scr[:, 0:32])
            # scrT row 0 = per-column maxes, row 1 = negated per-column mins
            nc.vector.tensor_reduce(
                scrT[0:2, 32:33], scrT[0:2, 0:32], axis=mybir.AxisListType.X,
                op=Alu.max,
            )
            bc = sb.tile([32, 2], f32)
            nc.vector.stream_shuffle(bc[:, 0:1], scrT[:, 32:33], mask=[0] * 32)
            nc.vector.stream_shuffle(bc[:, 1:2], scrT[:, 32:33], mask=[1] * 32)
            gmax = bc[0:D, 0:1]  # max(h) on every partition
            mneg = bc[0:D, 1:2]  # -min(h) on every partition

            # alpha = 0.6/(gmax+mneg)
            r1 = sb.tile([D, 1], f32)
            nc.vector.tensor_scalar(
                r1[:], gmax, mneg, 1.0 / 0.6, op0=Alu.add, op1=Alu.mult
            )
            alpha = sb.tile([D, 1], f32)
            nc.vector.reciprocal(alpha[:], r1[:])

            # dlt = (h+mneg)*alpha - 0.3 - h   (mapped minus h)
            p = sb.tile([D, RPC], f32)
            nc.vector.tensor_scalar(
                p[:], v_hTs, mneg, alpha[:], op0=Alu.add, op1=Alu.mult
            )
            dlt = sb.tile([D, RPC], f32)
            i_dlt = nc.vector.scalar_tensor_tensor(
                dlt[:], in0=p[:], scalar=0.3, in1=v_hTs,
                op0=Alu.subtract, op1=Alu.subtract,
            )

            # s = (max(gmax, mneg) > 100) fused into one tensor_scalar.
            # Ordered after dlt (nosync) so it fills the dw-matmul bubble
            # instead of delaying the alpha chain.
            s = sb.tile([D, 1], f32)
            i_s = nc.vector.tensor_scalar(
                s[:], gmax, mneg, 100.0, op0=Alu.max, op1=Alu.is_gt
            )
            add_dep_helper(i_s.ins, i_dlt.ins, sync=False)

            # ---- dw = dlt.T @ wp (correction matmul) ----
            dw = ps.tile([RPC, D], f32)
            nc.tensor.matmul(dw[:], dlt[:], v_wp, start=True, stop=True)

            # ---- Gumbel: b = ln(-ln(U + eps) + eps); g = -b (ACT) ----
            a1 = sb.tile([RPC, D], f32)
            nc.scalar.activation(a1[:], v_u1, Act.Ln, bias=eps_b, scale=1.0)
            a2 = sb.tile([RPC, D], f32)
            nc.scalar.activation(a2[:], v_u2, Act.Ln, bias=eps_b, scale=1.0)
            b1 = sb.tile([RPC, D], f32)
            nc.scalar.activation(b1[:], a1[:], Act.Ln, bias=eps_b, scale=-1.0)
            b2 = sb.tile([RPC, D], f32)
            nc.scalar.activation(b2[:], a2[:], Act.Ln, bias=eps_b, scale=-1.0)

            # base = bias + g1 - g2 = bias - b1 + b2.  Ordered after `dlt`
            # (nosync dep) so these fill the DVE bubble during the dw matmul
            # instead of delaying the critical chain.
            gg = sb.tile([RPC, D], f32)
            i_gg = nc.vector.tensor_sub(gg[:], b2[:], b1[:])
            add_dep_helper(i_gg.ins, i_dlt.ins, sync=False)
            base = sb.tile([RPC, D], f32)
            nc.vector.tensor_add(base[:], gg[:], v_bias)
            # l0 = base + pm0 overlaps the dw matmul tail (pm0 is long
            # done), so only the final STT waits on the dw semaphore
            l0 = sb.tile([RPC, D], f32)
            nc.vector.tensor_add(l0[:], base[:], pm0[:])

            # logits = s*dw + l0; sigmoid is monotonic so the top-5
            # threshold compare runs on logits directly
            logits = sb.tile([RPC, D], f32)
            nc.vector.scalar_tensor_tensor(
                logits[:], in0=dw[:], scalar=s[0:RPC, :], in1=l0[:],
                op0=Alu.mult, op1=Alu.add,
            )
            top8 = sb.tile([RPC, 8], f32)
            nc.vector.max(top8[:], logits[:])
            hard = sb.tile([RPC, D], f32)
            nc.vector.tensor_scalar(
                hard[:], logits[:], top8[:, 4:5], None, op0=Alu.is_ge
            )

            i_out = nc.sync.dma_start(out_s[:], hard[:], single_packet=True)

    # ---- overlap the out-DMA completion with the first TC-exit barrier --
    # TC exit emits: SP drain (waits all sems incl the out-DMA's) ->
    # barrier -> Pool reset-drain (waits the DMA again via its reset
    # range) + range-clear -> barrier.  Dropping the out-DMA wait from the
    # SP drain lets barrier 1 run during the ~1.4us HBM write receipt; the
    # Pool reset-drain still enforces completion before the clear.
    end_blk = next(b for b in nc.main_func.blocks if b.name.endswith("_end"))
    out_sem_id = next(
        u.id for u in i_out.ins.sync_info.on_update if u.update_value == 16
    )
    sp_drain = next(
        i for i in end_blk.instructions
        if type(i).__name__ == "InstDrain" and str(i.engine).endswith("SP")
    )
    pool_drain = next(
        i for i in end_blk.instructions
        if type(i).__name__ == "InstDrain" and i.is_reset_sema
    )
    assert pool_drain.reset_range_start <= out_sem_id < pool_drain.reset_range_stop
    # All of the SP drain's waits are transitively implied by the out-DMA
    # issue preceding it in SP program order (the issue required `hard`,
    # which required the whole ACT/PE/DVE chain); the out-DMA completion
    # itself is enforced by the Pool reset-drain.  Drop them all.
    old_waits = sp_drain.sync_info.on_wait
    assert any(x.id == out_sem_id for x in old_waits), (out_sem_id, old_waits)
    sp_drain.sync_info.on_wait = []

    # restore the manual semaphores so the NEFF is safely re-executable
    sem_lo = min(dma_sem.num, eps_sem.num)
    sem_hi = max(dma_sem.num, eps_sem.num)
    nc.gpsimd.dma_reset(range(sem_lo, sem_hi + 1))
    nc.gpsimd.sem_clear(range(sem_lo, sem_hi + 1))

    # ---- hoist the tagged pre-TC instructions into the engine preambles ---
    # each engine's init preamble ends with its bcreg1_hi register move;
    # inserting right after it puts the instruction before the codegen's
    # second sync point, so the DMA issue/memsets overlap the fixed
    # engine-boot phase.
    hoist_insts = list(hoist)
    idx = {id(inst): k for k, inst in enumerate(entry)}
    positions = sorted(idx[id(inst)] for inst in hoist_insts)
    for pos in reversed(positions):
        del entry[pos]

    def preamble_end(engine_prefix):
        for k, ins in enumerate(entry[:init_len]):
            if (
                type(ins).__name__ == "InstRegisterMove"
                and f"{engine_prefix}_bcreg1_hi" in str(ins)
            ):
                return k + 1
        raise RuntimeError(f"no preamble end for {engine_prefix}")

    by_engine = {}
    for inst in hoist_insts:
        by_engine.setdefault(str(inst.engine), []).append(inst)
    targets = []
    for eng_name, insts in by_engine.items():
        prefix = eng_name.split(".")[-1]  # EngineType.Activation -> Activation
        targets.append((preamble_end(prefix), insts))
    for pos, insts in sorted(targets, reverse=True):
        for inst in reversed(insts):
            entry.insert(pos, inst)

    nc.compile()
    return nc


def _get_nc():
    if "nc" not in _CACHE:
        _CACHE["nc"] = _build_nc()
    return _CACHE["nc"]


def _make_in_maps(h, w_p, bias, U1, U2):
    h = np.ascontiguousarray(np.asarray(h, np.float32).reshape(ROWS, D))
    hT = h.T
    wp = np.asarray(w_p, np.float32)
    bias = np.asarray(bias, np.float32).reshape(ROWS, D)
    u1 = np.asarray(U1, np.float32).reshape(ROWS, D)
    u2 = np.asarray(U2, np.float32).reshape(ROWS, D)

    in_maps = []
    for c in range(N_CORES):
        rows = slice(c * RPC, (c + 1) * RPC)
        pa = np.full((D, C_END), 0.5, np.float32)
        pa[:, C_HT:C_HTS] = hT
        pa[:, C_HTS:C_WP] = h[rows].T
        pa[:, C_WP:C_B] = wp
        pa[0:RPC, C_B:C_U1] = bias[rows]
        pa[0:RPC, C_U1:C_U2] = u1[rows]
        pa[0:RPC, C_U2:C_END] = u2[rows]
        in_maps.append({"packed": pa})
    return in_maps


def kernel(h, input, w_p, bias, U1, U2, **_unused):
    from concourse.bass_utils import run_bass_kernel_spmd

    nc = _get_nc()
    in_maps = _make_in_maps(h, w_p, bias, U1, U2)
    res = run_bass_kernel_spmd(nc, in_maps, core_ids=list(range(N_CORES)))
    out = np.concatenate([r["out_s"] for r in res.results], axis=0)
    return out.reshape(ROWS, 4, 4).astype(np.float32)
